# revision 5
# baseline (speedup 1.0000x reference)
"""Trainium2 Bass kernel for the DescriptorMatcher all-pairs problem.

Computes, for v1 = out1 reshaped [N1, C], v2 = out2 reshaped [N2, C]:
  out[n1*N2+n2, o] = sum_c v1[n1,c] * W[o,c] * v2[n2,c] + bias[o]
  out_norm[0, n1, n2] = || v1n[n1] - v2n[n2] ||,  vn = v / (eps + |v|)

Sharding: N1 rows split across 8 cores (128 rows each); v2 replicated.

Per-core device program (SPMD):
  - out head: two matmuls with K=65 (bias folded in as an extra
    contraction row against a row of ones in the rhs).
  - distance: G = v1 @ v2.T via K=64 matmul; r1 = 1/(eps+|v1|) per
    partition; r2 = 1/(eps+|v2|) as a [1, N2] row (norms via ones-vector
    matmul over squared v2), broadcast to [128, N2] with a rank-1 matmul;
    dist = sqrt(-2*r1*r2*G + 2.0)  (|vn|^2 ~ 1 to < 3e-7 rel).
"""

import numpy as np

C = 64
N1 = 1024
N2 = 1024
NCORES = 8
SLICE = N1 // NCORES  # 128
EPS = 1e-6

_CACHE = {}


def _build_program():
    import concourse.bacc as bacc
    import concourse.mybir as mybir
    import concourse.tile as tile
    from concourse._compat import get_trn_type

    fp32 = mybir.dt.float32
    AF = mybir.ActivationFunctionType
    OP = mybir.AluOpType

    nc = bacc.Bacc(get_trn_type() or "TRN2", target_bir_lowering=False, debug=False)

    x1 = nc.dram_tensor("x1", [C, SLICE], fp32, kind="ExternalInput").ap()
    x1t = nc.dram_tensor("x1t", [SLICE, C], fp32, kind="ExternalInput").ap()
    x2 = nc.dram_tensor("x2", [C, N2], fp32, kind="ExternalInput").ap()
    wc = nc.dram_tensor("wc", [C, 2], fp32, kind="ExternalInput").ap()
    brow = nc.dram_tensor("brow", [2, SLICE], fp32, kind="ExternalInput").ap()
    onesrow = nc.dram_tensor("onesrow", [1, N2], fp32, kind="ExternalInput").ap()
    onescol = nc.dram_tensor("onescol", [C, 1], fp32, kind="ExternalInput").ap()
    out_o = nc.dram_tensor("out_o", [SLICE, N2, 2], fp32, kind="ExternalOutput").ap()
    out_n = nc.dram_tensor("out_n", [SLICE, N2], fp32, kind="ExternalOutput").ap()

    with tile.TileContext(nc) as tc:
        with (
            tc.tile_pool(name="sb", bufs=1) as sb,
            tc.tile_pool(name="ps", bufs=1, space="PSUM") as ps,
            tc.tile_pool(name="psa", bufs=1, space="PSUM") as psa,
        ):
            # ---- inputs -> SBUF
            rhs = sb.tile([C + 1, N2], fp32)  # [v2 ; ones]
            nc.sync.dma_start(rhs[0:C, :], x2)
            nc.sync.dma_start(rhs[C : C + 1, :], onesrow)
            orow = sb.tile([1, SLICE], fp32)  # ones at partition 0 (bcast lhsT)
            nc.sync.dma_start(orow[:], onesrow[:, 0:SLICE])
            x1s = sb.tile([C, SLICE], fp32)
            nc.sync.dma_start(x1s[:], x1)
            x1ts = sb.tile([SLICE, C], fp32)
            nc.sync.dma_start(x1ts[:], x1t)
            wcs = sb.tile([C, 2], fp32)
            nc.sync.dma_start(wcs[:], wc)
            ocol = sb.tile([C, 1], fp32)
            nc.sync.dma_start(ocol[:], onescol)

            # ---- lhsT for the two output channels, bias as row C
            l0 = sb.tile([C + 1, SLICE], fp32)
            l1 = sb.tile([C + 1, SLICE], fp32)
            nc.sync.dma_start(l0[C : C + 1, :], brow[0:1, :])
            nc.sync.dma_start(l1[C : C + 1, :], brow[1:2, :])
            nc.vector.tensor_scalar_mul(l0[0:C, :], x1s[:], wcs[:, 0:1])
            nc.vector.tensor_scalar_mul(l1[0:C, :], x1s[:], wcs[:, 1:2])

            # ---- r1 chain (per-partition, fast)
            x1sq = sb.tile([SLICE, C], fp32)
            s1 = sb.tile([SLICE, 1], fp32)
            nc.vector.tensor_mul(x1sq[:], x1ts[:], x1ts[:])
            nc.vector.reduce_sum(s1[:], x1sq[:], axis=mybir.AxisListType.X)
            n1c = sb.tile([SLICE, 1], fp32)
            nc.scalar.sqrt(n1c[:], s1[:])
            n1e = sb.tile([SLICE, 1], fp32)
            nc.vector.tensor_scalar_add(n1e[:], n1c[:], EPS)
            r1 = sb.tile([SLICE, 1], fp32)
            nc.vector.reciprocal(r1[:], n1e[:])
            r1m2 = sb.tile([SLICE, 1], fp32)
            nc.vector.tensor_scalar_mul(r1m2[:], r1[:], -2.0)

            # ---- r2 row chain
            x2sq = sb.tile([C, N2], fp32)
            nc.vector.tensor_mul(x2sq[:], rhs[0:C, :], rhs[0:C, :])
            s2p = psa.tile([1, N2], fp32, tag="a")
            for j in range(2):
                nc.tensor.matmul(
                    s2p[:, j * 512 : (j + 1) * 512],
                    ocol[:],
                    x2sq[:, j * 512 : (j + 1) * 512],
                )
            n2row = sb.tile([1, N2], fp32)
            nc.scalar.sqrt(n2row[:], s2p[:])
            n2e = sb.tile([1, N2], fp32)
            nc.vector.tensor_scalar_add(n2e[:], n2row[:], EPS)
            r2row = sb.tile([1, N2], fp32)
            nc.vector.reciprocal(r2row[:], n2e[:])

            # broadcast r2 row to all partitions via rank-1 matmul
            r2Bp = psa.tile([SLICE, N2], fp32, tag="a")
            for j in range(2):
                nc.tensor.matmul(
                    r2Bp[:, j * 512 : (j + 1) * 512],
                    orow[:],
                    r2row[:, j * 512 : (j + 1) * 512],
                )
            r2Bs = sb.tile([SLICE, N2], fp32)
            nc.scalar.copy(r2Bs[:], r2Bp[:])

            # ---- main matmuls
            p0 = ps.tile([SLICE, N2], fp32, tag="p0")
            p1 = ps.tile([SLICE, N2], fp32, tag="p1")
            pg = ps.tile([SLICE, N2], fp32, tag="pg")
            for j in range(2):
                sl = slice(j * 512, (j + 1) * 512)
                nc.tensor.matmul(p0[:, sl], l0[:], rhs[:, sl])
                nc.tensor.matmul(p1[:, sl], l1[:], rhs[:, sl])
                nc.tensor.matmul(pg[:, sl], x1s[:], rhs[0:C, sl])

            # ---- out head: interleave channels (bias already added)
            outsb = sb.tile([SLICE, N2, 2], fp32)
            nc.vector.tensor_copy(outsb[:, :, 0], p0[:])
            nc.scalar.copy(outsb[:, :, 1], p1[:])
            nc.sync.dma_start(out_o, outsb[:])

            # ---- distance: sqrt(-2*r1*r2*G + 2)
            d2 = sb.tile([SLICE, N2], fp32)
            nc.vector.scalar_tensor_tensor(
                out=d2[:],
                in0=pg[:],
                scalar=r1m2[:],
                in1=r2Bs[:],
                op0=OP.mult,
                op1=OP.mult,
            )
            two = sb.tile([SLICE, 1], fp32)
            nc.vector.memset(two[:], 2.0)
            dist = sb.tile([SLICE, N2], fp32)
            nc.scalar.activation(dist[:], d2[:], AF.Sqrt, bias=two[:], scale=1.0)
            nc.sync.dma_start(out_n, dist[:])

    nc.compile()
    return nc


def _get_program():
    if "nc" not in _CACHE:
        _CACHE["nc"] = _build_program()
    return _CACHE["nc"]


def make_in_maps(out1, out2, W, bias):
    v1 = np.ascontiguousarray(out1.reshape(C, N1), dtype=np.float32)
    v2 = np.ascontiguousarray(out2.reshape(C, N2), dtype=np.float32)
    wc = np.ascontiguousarray(W.astype(np.float32).T)  # [C, 2]
    brow = np.ascontiguousarray(
        np.repeat(bias.astype(np.float32)[:, None], SLICE, axis=1)
    )  # [2, SLICE]
    onesrow = np.ones((1, N2), dtype=np.float32)
    onescol = np.ones((C, 1), dtype=np.float32)
    in_maps = []
    for k in range(NCORES):
        x1 = np.ascontiguousarray(v1[:, k * SLICE : (k + 1) * SLICE])
        in_maps.append(
            {
                "x1": x1,
                "x1t": np.ascontiguousarray(x1.T),
                "x2": v2,
                "wc": wc,
                "brow": brow,
                "onesrow": onesrow,
                "onescol": onescol,
            }
        )
    return in_maps


def gather_results(results):
    out = np.concatenate(
        [results[k]["out_o"].reshape(SLICE * N2, 2) for k in range(NCORES)], axis=0
    )
    out_norm = np.concatenate([results[k]["out_n"] for k in range(NCORES)], axis=0)[
        None, :, :
    ]
    return out, out_norm


def kernel(out1, out2, W, bias):
    from concourse.bass_utils import run_bass_kernel_spmd

    nc = _get_program()
    in_maps = make_in_maps(
        np.asarray(out1), np.asarray(out2), np.asarray(W), np.asarray(bias)
    )
    res = run_bass_kernel_spmd(nc, in_maps, list(range(NCORES)))
    return gather_results(res.results)


# revision 8
# speedup vs baseline: 1.1206x; 1.1206x over previous
"""Trainium2 Bass kernel for the DescriptorMatcher all-pairs problem.

Reference semantics (v1 = out1 as [N1, C], v2 = out2 as [N2, C]):
  out[n1*N2+n2, o]   = sum_c v1[n1,c] * W[o,c] * v2[n2,c] + bias[o]
  out_norm[0,n1,n2]  = || v1/(eps+|v1|) - v2/(eps+|v2|) ||
                     = sqrt(2 - 2*r1[n1]*r2[n2]*G[n1,n2])  (+O(3e-7))
  with G = v1 @ v2.T and r = 1/(eps+|v|).

Sharding: N1 split across 8 cores (128 rows each), v2 replicated.

Device program highlights:
  - All big matmuls in bf16 with 3-way mantissa splits (h+l1+l2 covers
    f32 precision) K-stacked in pairs: per [128,1024] output only three
    K=128 bf16 passes:  [h;l1]@[yh;yh] + [h;l1]@[yl1;yl1] + [h;l2]@[yl2;yh]
    (error ~2^-26, matmul cost scales with N not K).
  - r1/r2 chains in column orientation (fast, 128 lanes) with one
    Newton step to fix the ACT Sqrt table error (~7e-6 -> ~1e-7).
  - r2 row produced by PE transpose + SBUF->SBUF DMA reshape, broadcast
    to [128, N2] by a rank-1 f32 matmul.
  - dist = ACT_Sqrt((G_psum * -2r1) * r2B + 2.0) via one DVE
    scalar_tensor_tensor + one ACT activation.
"""

import numpy as np

C = 64
N1 = 1024
N2 = 1024
NCORES = 8
SLICE = N1 // NCORES  # 128
EPS = 1e-6
H = 512  # N-half

_CACHE = {}


def _split3_np(x):
    import ml_dtypes

    bf = ml_dtypes.bfloat16
    h = x.astype(bf)
    r1 = (x - h.astype(np.float32)).astype(np.float32)
    l1 = r1.astype(bf)
    l2 = (r1 - l1.astype(np.float32)).astype(bf)
    return h, l1, l2


def _build_program():
    import concourse.bacc as bacc
    import concourse.mybir as mybir
    import concourse.tile as tile
    from concourse._compat import get_trn_type

    fp32 = mybir.dt.float32
    bf16 = mybir.dt.bfloat16
    AF = mybir.ActivationFunctionType
    OP = mybir.AluOpType
    X = mybir.AxisListType.X

    nc = bacc.Bacc(get_trn_type() or "TRN2", target_bir_lowering=False, debug=False)

    # ---- DRAM I/O (packed)
    # f32a: x1 c-major | wc              [64, 130]
    f32a = nc.dram_tensor("f32a", [C, SLICE + 2], fp32, kind="ExternalInput").ap()
    # rstk: x2 K-stacked bf16 rhs: [x2h;x2h] | [x2l1;x2l1] | [x2l2;x2h]
    rstk = nc.dram_tensor("rstk", [2 * C, 3 * N2], bf16, kind="ExternalInput").ap()
    # x1bf: x1h | x1l1 | x1l2            [64, 384]
    x1bf = nc.dram_tensor("x1bf", [C, 3 * SLICE], bf16, kind="ExternalInput").ap()
    # f32d: x1t | identity | x2t | bcol  [128, 64+128+512+2]
    f32d = nc.dram_tensor("f32d", [SLICE, C + 128 + 512 + 2], fp32,
                          kind="ExternalInput").ap()
    out_o = nc.dram_tensor("out_o", [SLICE, N2, 2], fp32, kind="ExternalOutput").ap()
    out_n = nc.dram_tensor("out_n", [SLICE, N2], fp32, kind="ExternalOutput").ap()

    with tile.TileContext(nc) as tc:
        with (
            tc.tile_pool(name="sb", bufs=1) as sb,
            tc.tile_pool(name="ps", bufs=1, space="PSUM") as ps,
        ):
            # ================= input DMAs (two queues) =================
            tf32a = sb.tile([C, SLICE + 2], fp32)
            nc.sync.dma_start(tf32a[:], f32a)
            trstk = sb.tile([2 * C, 3 * N2], bf16)
            nc.scalar.dma_start(trstk[:], rstk)
            tx1bf = sb.tile([C, 3 * SLICE], bf16)
            nc.sync.dma_start(tx1bf[:], x1bf)
            tf32d = sb.tile([SLICE, C + 128 + 512 + 2], fp32)
            nc.sync.dma_start(tf32d[:], f32d)

            x1f = tf32a[:, 0:SLICE]
            wcs = tf32a[:, SLICE : SLICE + 2]
            x1t = tf32d[:, 0:C]
            ident = tf32d[:, C : C + 128]
            x2t = tf32d[:, C + 128 : C + 128 + 512]  # [128, 8, 64] flat
            bcol = tf32d[:, C + 128 + 512 : C + 128 + 512 + 2]
            R_hh = trstk[:, 0:N2]
            R_l1 = trstk[:, N2 : 2 * N2]
            R_mx = trstk[:, 2 * N2 : 3 * N2]

            # ================= lhsT preps =================
            # pg lhsT stacks from host-provided x1 splits
            g_s1 = sb.tile([2 * C, SLICE], bf16)  # [x1h; x1l1]
            nc.vector.tensor_copy(g_s1[0:C, :], tx1bf[:, 0:SLICE])
            nc.vector.tensor_copy(g_s1[C:, :], tx1bf[:, SLICE : 2 * SLICE])
            g_s2 = sb.tile([2 * C, SLICE], bf16)  # [x1h; x1l2]
            nc.vector.tensor_copy(g_s2[0:C, :], tx1bf[:, 0:SLICE])
            nc.vector.tensor_copy(g_s2[C:, :], tx1bf[:, 2 * SLICE : 3 * SLICE])

            # out-head lhsTs: l{o}f = x1f * W[o], split 3-way on device
            heads = []
            for o in range(2):
                lf = sb.tile([C, SLICE], fp32, tag=f"lf{o}")
                nc.vector.tensor_scalar_mul(lf[:], x1f, wcs[:, o : o + 1])
                s1t = sb.tile([2 * C, SLICE], bf16, tag=f"hs1{o}")  # [h; l1]
                s2t = sb.tile([2 * C, SLICE], bf16, tag=f"hs2{o}")  # [h; l2]
                nc.scalar.copy(s1t[0:C, :], lf[:])  # h = bf16(lf)
                nc.scalar.copy(s2t[0:C, :], s1t[0:C, :])
                r1f = sb.tile([C, SLICE], fp32, tag=f"r1f{o}")
                nc.vector.tensor_tensor(
                    out=r1f[:], in0=lf[:], in1=s1t[0:C, :], op=OP.subtract
                )
                l1b = sb.tile([C, SLICE], bf16, tag=f"l1b{o}")
                nc.scalar.copy(l1b[:], r1f[:])  # l1 = bf16(r), base-0 copy
                nc.scalar.copy(s1t[C:, :], l1b[:])
                r2f = sb.tile([C, SLICE], fp32, tag=f"r2f{o}")
                nc.vector.tensor_tensor(
                    out=r2f[:], in0=r1f[:], in1=l1b[:], op=OP.subtract
                )
                nc.scalar.copy(s2t[C:, :], r2f[:])  # l2
                heads.append((s1t, s2t))

            # ================= r1 chain (col, [128,1]) =================
            x1sq = sb.tile([SLICE, C], fp32)
            nc.vector.tensor_mul(x1sq[:], x1t, x1t)
            s1c = sb.tile([SLICE, 1], fp32)
            nc.vector.reduce_sum(s1c[:], x1sq[:], axis=X)
            a1 = sb.tile([SLICE, 1], fp32)
            nc.scalar.sqrt(a1[:], s1c[:])
            i1 = sb.tile([SLICE, 1], fp32)
            nc.vector.reciprocal(i1[:], a1[:])
            t1 = sb.tile([SLICE, 1], fp32)
            nc.vector.tensor_mul(t1[:], s1c[:], i1[:])
            u1 = sb.tile([SLICE, 1], fp32)
            nc.vector.tensor_add(u1[:], a1[:], t1[:])
            n1c = sb.tile([SLICE, 1], fp32)
            nc.vector.tensor_scalar(
                out=n1c[:], in0=u1[:], scalar1=0.5, scalar2=EPS,
                op0=OP.mult, op1=OP.add,
            )
            r1c = sb.tile([SLICE, 1], fp32)
            nc.vector.reciprocal(r1c[:], n1c[:])
            r1m2 = sb.tile([SLICE, 1], fp32)
            nc.vector.tensor_scalar_mul(r1m2[:], r1c[:], -2.0)

            # ================= r2 chain (col, [128,8]) =================
            x2sq = sb.tile([SLICE, 512], fp32)
            nc.vector.tensor_mul(x2sq[:], x2t, x2t)
            s2c = sb.tile([SLICE, 8], fp32)
            nc.vector.reduce_sum(
                s2c[:], x2sq[:].rearrange("p (t c) -> p t c", c=C), axis=X
            )
            a2 = sb.tile([SLICE, 8], fp32)
            nc.scalar.sqrt(a2[:], s2c[:])
            i2 = sb.tile([SLICE, 8], fp32)
            nc.vector.reciprocal(i2[:], a2[:])
            t2 = sb.tile([SLICE, 8], fp32)
            nc.vector.tensor_mul(t2[:], s2c[:], i2[:])
            u2 = sb.tile([SLICE, 8], fp32)
            nc.vector.tensor_add(u2[:], a2[:], t2[:])
            n2c = sb.tile([SLICE, 8], fp32)
            nc.vector.tensor_scalar(
                out=n2c[:], in0=u2[:], scalar1=0.5, scalar2=EPS,
                op0=OP.mult, op1=OP.add,
            )
            r2c8 = sb.tile([SLICE, 8], fp32)
            nc.vector.reciprocal(r2c8[:], n2c[:])

            # transpose [128,8] -> [8,128], reshape to [1,1024] via DMA
            ptr = ps.tile([8, 128], fp32, tag="rb")
            nc.tensor.transpose(ptr[:], r2c8[:], ident)
            r2t8 = sb.tile([8, 128], fp32)
            nc.vector.tensor_copy(r2t8[:], ptr[:])
            r2row = sb.tile([1, N2], fp32)
            nc.sync.dma_start(r2row[:], r2t8[:])
            orow = sb.tile([1, SLICE], fp32)
            nc.vector.memset(orow[:], 1.0)

            # ================= big matmuls =================
            pg = ps.tile([SLICE, N2], fp32, tag="pg")
            p0 = ps.tile([SLICE, N2], fp32, tag="p0")
            p1 = ps.tile([SLICE, N2], fp32, tag="p1")
            for j in range(2):
                sl = slice(j * H, (j + 1) * H)
                nc.tensor.matmul(pg[:, sl], g_s1[:], R_hh[:, sl], start=True, stop=False)
                nc.tensor.matmul(pg[:, sl], g_s1[:], R_l1[:, sl], start=False, stop=False)
                nc.tensor.matmul(pg[:, sl], g_s2[:], R_mx[:, sl],
                                 start=False, stop=True)
            for j in range(2):
                sl = slice(j * H, (j + 1) * H)
                for o, pt in ((0, p0), (1, p1)):
                    s1t, s2t = heads[o]
                    nc.tensor.matmul(pt[:, sl], s1t[:], R_hh[:, sl], start=True, stop=False)
                    nc.tensor.matmul(pt[:, sl], s1t[:], R_l1[:, sl], start=False, stop=False)
                    nc.tensor.matmul(pt[:, sl], s2t[:], R_mx[:, sl],
                                     start=False, stop=True)

            # r2 broadcast via rank-1 f32 matmul (after ptr released)
            r2B = ps.tile([SLICE, N2], fp32, tag="rb")
            for j in range(2):
                sl = slice(j * H, (j + 1) * H)
                nc.tensor.matmul(r2B[:, sl], orow[:], r2row[:, sl])

            # ================= outputs (halves for pipelining) =========
            outsb = sb.tile([SLICE, N2, 2], fp32)
            r2Bs = sb.tile([SLICE, N2], fp32)
            d2 = sb.tile([SLICE, N2], fp32)
            dist = sb.tile([SLICE, N2], fp32)
            two = sb.tile([SLICE, 1], fp32)
            nc.vector.memset(two[:], 2.0)
            for j in range(2):
                sl = slice(j * H, (j + 1) * H)
                nc.scalar.copy(r2Bs[:, sl], r2B[:, sl])
                nc.vector.scalar_tensor_tensor(
                    out=d2[:, sl], in0=pg[:, sl], scalar=r1m2[:],
                    in1=r2Bs[:, sl], op0=OP.mult, op1=OP.mult,
                )
                nc.scalar.activation(
                    dist[:, sl], d2[:, sl], AF.Sqrt, bias=two[:], scale=1.0
                )
                nc.sync.dma_start(out_n[:, sl], dist[:, sl])
            for j in range(2):
                sl = slice(j * H, (j + 1) * H)
                nc.vector.tensor_scalar_add(
                    outsb[:, sl, 0], p0[:, sl], bcol[:, 0:1]
                )
                nc.scalar.activation(
                    outsb[:, sl, 1], p1[:, sl], AF.Identity,
                    bias=bcol[:, 1:2], scale=1.0,
                )
                nc.scalar.dma_start(out_o[:, sl, :], outsb[:, sl, :])

    nc.compile()
    return nc


def _get_program():
    if "nc" not in _CACHE:
        _CACHE["nc"] = _build_program()
    return _CACHE["nc"]


def make_in_maps(out1, out2, W, bias):
    v1 = np.ascontiguousarray(out1.reshape(C, N1), dtype=np.float32)
    v2 = np.ascontiguousarray(out2.reshape(C, N2), dtype=np.float32)
    W = np.asarray(W, dtype=np.float32)
    bias = np.asarray(bias, dtype=np.float32)

    x2h, x2l1, x2l2 = _split3_np(v2)
    rstk = np.concatenate(
        [
            np.concatenate([x2h, x2h], axis=0),
            np.concatenate([x2l1, x2l1], axis=0),
            np.concatenate([x2l2, x2h], axis=0),
        ],
        axis=1,
    )  # [128, 3*N2] bf16
    rstk = np.ascontiguousarray(rstk)

    ident = np.eye(128, dtype=np.float32)
    # v2 col-major tiles for the s2 chain: x2t[p, t, c] = v2[c, t*128+p]
    v2t = v2.T.reshape(8, 128, C).transpose(1, 0, 2).reshape(128, 512)
    bcolfull = np.repeat(bias[None, :], 128, axis=0)  # [128, 2]

    in_maps = []
    for k in range(NCORES):
        x1 = np.ascontiguousarray(v1[:, k * SLICE : (k + 1) * SLICE])
        h, l1, l2 = _split3_np(x1)
        f32a = np.ascontiguousarray(np.concatenate([x1, W.T], axis=1))
        x1bf = np.ascontiguousarray(np.concatenate([h, l1, l2], axis=1))
        f32d = np.ascontiguousarray(
            np.concatenate([x1.T, ident, v2t, bcolfull], axis=1)
        )
        in_maps.append({"f32a": f32a, "rstk": rstk, "x1bf": x1bf, "f32d": f32d})
    return in_maps


def gather_results(results):
    out = np.concatenate(
        [results[k]["out_o"].reshape(SLICE * N2, 2) for k in range(NCORES)], axis=0
    )
    out_norm = np.concatenate([results[k]["out_n"] for k in range(NCORES)], axis=0)[
        None, :, :
    ]
    return out, out_norm


def kernel(out1, out2, W, bias):
    from concourse.bass_utils import run_bass_kernel_spmd

    nc = _get_program()
    in_maps = make_in_maps(
        np.asarray(out1), np.asarray(out2), np.asarray(W), np.asarray(bias)
    )
    res = run_bass_kernel_spmd(nc, in_maps, list(range(NCORES)))
    return gather_results(res.results)


# revision 9
# speedup vs baseline: 1.2218x; 1.0904x over previous
"""Trainium2 Bass kernel for the DescriptorMatcher all-pairs problem.

Reference semantics (v1 = out1 as [N1, C], v2 = out2 as [N2, C]):
  out[n1*N2+n2, o]   = sum_c v1[n1,c] * W[o,c] * v2[n2,c] + bias[o]
  out_norm[0,n1,n2]  = || v1/(eps+|v1|) - v2/(eps+|v2|) ||
                     = sqrt(2 - 2*r1[n1]*r2[n2]*G[n1,n2])  (+O(3e-7))
  with G = v1 @ v2.T and r = 1/(eps+|v|).

Sharding: N1 split across 8 cores (128 rows each), v2 replicated.

Device program highlights:
  - Big matmuls in bf16 with mantissa splits (h+l1+l2 covers f32)
    K-stacked in pairs; per [128,1024] output three K=128 bf16 passes:
    [h;l1]@[yh;yh] + [h;l1]@[yl1;yl1] + [h;l2]@[yl2;yh]  (err ~2^-26).
    The out-head optionally runs 2-pass (drops the l2 cross terms).
  - r1/r2 norm chains in column orientation (128 lanes) with one Newton
    step to fix the ACT Sqrt table error (~7e-6 -> ~1e-7).
  - r2 row: 3-way bf16 split in columns, one PE transpose [128,24] ->
    [24,128], SBUF->SBUF cast-DMA reshape to [3,1024] bf16, then ONE
    K=3 ones-matmul sums the components while broadcasting -> r2B.
  - dist = ACT_Sqrt((G_psum * -2r1) * r2B + 2.0).
"""

import numpy as np

C = 64
N1 = 1024
N2 = 1024
NCORES = 8
SLICE = N1 // NCORES  # 128
EPS = 1e-6
H = 512  # N-half
HEAD_3PASS = False  # True: exact-ish out head (3 bf16 passes per channel)

_CACHE = {}


def _split3_np(x):
    import ml_dtypes

    bf = ml_dtypes.bfloat16
    h = x.astype(bf)
    r1 = (x - h.astype(np.float32)).astype(np.float32)
    l1 = r1.astype(bf)
    l2 = (r1 - l1.astype(np.float32)).astype(bf)
    return h, l1, l2


def _build_program():
    import concourse.bacc as bacc
    import concourse.mybir as mybir
    import concourse.tile as tile
    from concourse._compat import get_trn_type

    fp32 = mybir.dt.float32
    bf16 = mybir.dt.bfloat16
    AF = mybir.ActivationFunctionType
    OP = mybir.AluOpType
    X = mybir.AxisListType.X

    nc = bacc.Bacc(get_trn_type() or "TRN2", target_bir_lowering=False, debug=False)

    # ---- DRAM I/O (packed into 3 inputs)
    # f32d: x1t | identity | x2t | bcol   [128, 64+128+512+2] f32
    f32d = nc.dram_tensor(
        "f32d", [SLICE, C + 128 + 512 + 2], fp32, kind="ExternalInput"
    ).ap()
    # x1pk (bf16): x1-f32-bitcast(260) | x1h | x1l1 | x1l2   [64, 260+384]
    x1pk = nc.dram_tensor(
        "x1pk", [C, 2 * (SLICE + 2) + 3 * SLICE], bf16, kind="ExternalInput"
    ).ap()
    # rstk: x2 K-stacked bf16 rhs: [x2h;x2h] | [x2l1;x2l1] | [x2l2;x2h]
    rstk = nc.dram_tensor("rstk", [2 * C, 3 * N2], bf16, kind="ExternalInput").ap()
    out_o = nc.dram_tensor("out_o", [SLICE, N2, 2], fp32, kind="ExternalOutput").ap()
    out_n = nc.dram_tensor("out_n", [SLICE, N2], fp32, kind="ExternalOutput").ap()

    with tile.TileContext(nc) as tc:
        with (
            tc.tile_pool(name="sb", bufs=1) as sb,
            tc.tile_pool(name="ps", bufs=1, space="PSUM") as ps,
        ):
            # ================= input DMAs =================
            tf32d = sb.tile([SLICE, C + 128 + 512 + 2], fp32)
            nc.sync.dma_start(tf32d[:], f32d)
            tx1pk = sb.tile([C, 2 * (SLICE + 2) + 3 * SLICE], bf16)
            nc.sync.dma_start(tx1pk[:], x1pk)
            trstk = sb.tile([2 * C, 3 * N2], bf16)
            nc.scalar.dma_start(trstk[:], rstk)

            x1t = tf32d[:, 0:C]
            ident = tf32d[:, C : C + 128]
            x2t = tf32d[:, C + 128 : C + 128 + 512]  # [128, 8*64] flat
            bcol = tf32d[:, C + 128 + 512 : C + 128 + 512 + 2]
            x1f32 = tx1pk[:, 0 : 2 * (SLICE + 2)].bitcast(fp32)  # [64, 130]
            x1f = x1f32[:, 0:SLICE]
            wcs = x1f32[:, SLICE : SLICE + 2]
            o1 = 2 * (SLICE + 2)
            x1h = tx1pk[:, o1 : o1 + SLICE]
            x1l1 = tx1pk[:, o1 + SLICE : o1 + 2 * SLICE]
            x1l2 = tx1pk[:, o1 + 2 * SLICE : o1 + 3 * SLICE]
            R_hh = trstk[:, 0:N2]
            R_l1 = trstk[:, N2 : 2 * N2]
            R_mx = trstk[:, 2 * N2 : 3 * N2]

            # ================= r2 chain (col, [128,8]) =================
            x2sq = sb.tile([SLICE, 512], fp32)
            nc.vector.tensor_mul(x2sq[:], x2t, x2t)
            s2c = sb.tile([SLICE, 8], fp32)
            nc.vector.reduce_sum(
                s2c[:], x2sq[:].rearrange("p (t c) -> p t c", c=C), axis=X
            )
            a2 = sb.tile([SLICE, 8], fp32)
            nc.scalar.sqrt(a2[:], s2c[:])
            i2 = sb.tile([SLICE, 8], fp32)
            nc.vector.reciprocal(i2[:], a2[:])
            t2 = sb.tile([SLICE, 8], fp32)
            nc.vector.tensor_mul(t2[:], s2c[:], i2[:])
            u2 = sb.tile([SLICE, 8], fp32)
            nc.vector.tensor_add(u2[:], a2[:], t2[:])
            n2c = sb.tile([SLICE, 8], fp32)
            nc.vector.tensor_scalar(
                out=n2c[:], in0=u2[:], scalar1=0.5, scalar2=EPS,
                op0=OP.mult, op1=OP.add,
            )
            r2c8 = sb.tile([SLICE, 8], fp32)
            nc.vector.reciprocal(r2c8[:], n2c[:])
            # 3-way bf16 split (values held in f32), packed [128, 24]
            T24 = sb.tile([SLICE, 24], fp32)
            bsc = sb.tile([SLICE, 8], bf16)
            nc.scalar.copy(bsc[:], r2c8[:])
            nc.scalar.copy(T24[:, 0:8], bsc[:])
            res1 = sb.tile([SLICE, 8], fp32)
            nc.vector.tensor_tensor(
                out=res1[:], in0=r2c8[:], in1=T24[:, 0:8], op=OP.subtract
            )
            b1c = sb.tile([SLICE, 8], bf16)
            nc.scalar.copy(b1c[:], res1[:])
            nc.scalar.copy(T24[:, 8:16], b1c[:])
            nc.vector.tensor_tensor(
                out=T24[:, 16:24], in0=res1[:], in1=T24[:, 8:16], op=OP.subtract
            )
            # transpose -> [24,128] psum -> sbuf -> cast reshape [3,1024] bf16
            ptr = ps.tile([24, 128], fp32, tag="rb")
            nc.tensor.transpose(ptr[:], T24[:], ident)
            r2t24 = sb.tile([24, 128], fp32)
            nc.vector.tensor_copy(r2t24[:], ptr[:])
            r2row3 = sb.tile([3, N2], bf16)
            nc.gpsimd.dma_start(r2row3[:], r2t24[:])  # SWDGE casts f32->bf16
            ones3 = sb.tile([3, SLICE], bf16)
            nc.vector.memset(ones3[:], 1.0)

            # ================= r1 chain (col, [128,1]) =================
            x1sq = sb.tile([SLICE, C], fp32)
            nc.vector.tensor_mul(x1sq[:], x1t, x1t)
            s1c = sb.tile([SLICE, 1], fp32)
            nc.vector.reduce_sum(s1c[:], x1sq[:], axis=X)
            a1 = sb.tile([SLICE, 1], fp32)
            nc.scalar.sqrt(a1[:], s1c[:])
            i1 = sb.tile([SLICE, 1], fp32)
            nc.vector.reciprocal(i1[:], a1[:])
            t1 = sb.tile([SLICE, 1], fp32)
            nc.vector.tensor_mul(t1[:], s1c[:], i1[:])
            u1 = sb.tile([SLICE, 1], fp32)
            nc.vector.tensor_add(u1[:], a1[:], t1[:])
            n1c = sb.tile([SLICE, 1], fp32)
            nc.vector.tensor_scalar(
                out=n1c[:], in0=u1[:], scalar1=0.5, scalar2=EPS,
                op0=OP.mult, op1=OP.add,
            )
            r1c = sb.tile([SLICE, 1], fp32)
            nc.vector.reciprocal(r1c[:], n1c[:])
            r1m2 = sb.tile([SLICE, 1], fp32)
            nc.vector.tensor_scalar_mul(r1m2[:], r1c[:], -2.0)

            # ================= lhsT preps =================
            g_s1 = sb.tile([2 * C, SLICE], bf16)  # [x1h; x1l1]
            nc.vector.tensor_copy(g_s1[0:C, :], x1h)
            nc.vector.tensor_copy(g_s1[C:, :], x1l1)
            g_s2 = sb.tile([2 * C, SLICE], bf16)  # [x1h; x1l2]
            nc.vector.tensor_copy(g_s2[0:C, :], x1h)
            nc.vector.tensor_copy(g_s2[C:, :], x1l2)

            heads = []
            for o in range(2):
                lf = sb.tile([C, SLICE], fp32, tag=f"lf{o}")
                nc.vector.tensor_scalar_mul(lf[:], x1f, wcs[:, o : o + 1])
                s1t = sb.tile([2 * C, SLICE], bf16, tag=f"hs1{o}")  # [h; l1]
                nc.scalar.copy(s1t[0:C, :], lf[:])
                l1b = sb.tile([C, SLICE], bf16, tag=f"l1b{o}")
                r1f = sb.tile([C, SLICE], fp32, tag=f"r1f{o}")
                nc.vector.tensor_tensor(
                    out=r1f[:], in0=lf[:], in1=s1t[0:C, :], op=OP.subtract
                )
                nc.scalar.copy(l1b[:], r1f[:])
                nc.scalar.copy(s1t[C:, :], l1b[:])
                if HEAD_3PASS:
                    s2t = sb.tile([2 * C, SLICE], bf16, tag=f"hs2{o}")  # [h; l2]
                    nc.scalar.copy(s2t[0:C, :], s1t[0:C, :])
                    r2f = sb.tile([C, SLICE], fp32, tag=f"r2f{o}")
                    nc.vector.tensor_tensor(
                        out=r2f[:], in0=r1f[:], in1=l1b[:], op=OP.subtract
                    )
                    nc.scalar.copy(s2t[C:, :], r2f[:])
                else:
                    s2t = None
                heads.append((s1t, s2t))

            # ================= big matmuls =================
            pg = ps.tile([SLICE, N2], fp32, tag="pg")
            p0 = ps.tile([SLICE, N2], fp32, tag="p0")
            p1 = ps.tile([SLICE, N2], fp32, tag="p1")
            for j in range(2):
                sl = slice(j * H, (j + 1) * H)
                nc.tensor.matmul(pg[:, sl], g_s1[:], R_hh[:, sl],
                                 start=True, stop=False)
                nc.tensor.matmul(pg[:, sl], g_s1[:], R_l1[:, sl],
                                 start=False, stop=False)
                nc.tensor.matmul(pg[:, sl], g_s2[:], R_mx[:, sl],
                                 start=False, stop=True)

            # r2 broadcast: ONE K=3 bf16 matmul per half sums h+l1+l2
            r2B = ps.tile([SLICE, N2], fp32, tag="rb")
            for j in range(2):
                sl = slice(j * H, (j + 1) * H)
                nc.tensor.matmul(r2B[:, sl], ones3[:], r2row3[:, sl])

            for j in range(2):
                sl = slice(j * H, (j + 1) * H)
                for o, pt in ((0, p0), (1, p1)):
                    s1t, s2t = heads[o]
                    if HEAD_3PASS:
                        nc.tensor.matmul(pt[:, sl], s1t[:], R_hh[:, sl],
                                         start=True, stop=False)
                        nc.tensor.matmul(pt[:, sl], s1t[:], R_l1[:, sl],
                                         start=False, stop=False)
                        nc.tensor.matmul(pt[:, sl], s2t[:], R_mx[:, sl],
                                         start=False, stop=True)
                    else:
                        nc.tensor.matmul(pt[:, sl], s1t[:], R_hh[:, sl],
                                         start=True, stop=False)
                        nc.tensor.matmul(pt[:, sl], s1t[:], R_l1[:, sl],
                                         start=False, stop=True)

            # ================= outputs (halves, pipelined) =========
            r2Bs = sb.tile([SLICE, N2], fp32)
            d2 = sb.tile([SLICE, N2], fp32)
            dist = sb.tile([SLICE, N2], fp32)
            two = sb.tile([SLICE, 1], fp32)
            nc.vector.memset(two[:], 2.0)
            outsb = sb.tile([SLICE, N2, 2], fp32)
            for j in range(2):
                sl = slice(j * H, (j + 1) * H)
                nc.scalar.copy(r2Bs[:, sl], r2B[:, sl])
                nc.vector.scalar_tensor_tensor(
                    out=d2[:, sl], in0=pg[:, sl], scalar=r1m2[:],
                    in1=r2Bs[:, sl], op0=OP.mult, op1=OP.mult,
                )
                nc.scalar.activation(
                    dist[:, sl], d2[:, sl], AF.Sqrt, bias=two[:], scale=1.0
                )
                nc.sync.dma_start(out_n[:, sl], dist[:, sl])
            for j in range(2):
                sl = slice(j * H, (j + 1) * H)
                nc.vector.tensor_scalar_add(outsb[:, sl, 0], p0[:, sl], bcol[:, 0:1])
                nc.scalar.activation(
                    outsb[:, sl, 1], p1[:, sl], AF.Identity,
                    bias=bcol[:, 1:2], scale=1.0,
                )
                nc.scalar.dma_start(out_o[:, sl, :], outsb[:, sl, :])

    nc.compile()
    return nc


def _get_program():
    if "nc" not in _CACHE:
        _CACHE["nc"] = _build_program()
    return _CACHE["nc"]


def make_in_maps(out1, out2, W, bias):
    import ml_dtypes

    bf = ml_dtypes.bfloat16
    v1 = np.ascontiguousarray(out1.reshape(C, N1), dtype=np.float32)
    v2 = np.ascontiguousarray(out2.reshape(C, N2), dtype=np.float32)
    W = np.asarray(W, dtype=np.float32)
    bias = np.asarray(bias, dtype=np.float32)

    x2h, x2l1, x2l2 = _split3_np(v2)
    rstk = np.ascontiguousarray(
        np.concatenate(
            [
                np.concatenate([x2h, x2h], axis=0),
                np.concatenate([x2l1, x2l1], axis=0),
                np.concatenate([x2l2, x2h], axis=0),
            ],
            axis=1,
        )
    )

    ident = np.eye(128, dtype=np.float32)
    v2t = v2.T.reshape(8, 128, C).transpose(1, 0, 2).reshape(128, 512)
    bcolfull = np.repeat(bias[None, :], 128, axis=0)

    in_maps = []
    for k in range(NCORES):
        x1 = np.ascontiguousarray(v1[:, k * SLICE : (k + 1) * SLICE])
        h, l1, l2 = _split3_np(x1)
        f32part = np.concatenate([x1, W.T], axis=1)  # [64, 130] f32
        x1pk = np.ascontiguousarray(
            np.concatenate(
                [f32part.view(bf).reshape(C, -1), h, l1, l2], axis=1
            )
        )
        f32d = np.ascontiguousarray(
            np.concatenate([x1.T, ident, v2t, bcolfull], axis=1)
        )
        in_maps.append({"f32d": f32d, "x1pk": x1pk, "rstk": rstk})
    return in_maps


def gather_results(results):
    out = np.concatenate(
        [results[k]["out_o"].reshape(SLICE * N2, 2) for k in range(NCORES)], axis=0
    )
    out_norm = np.concatenate([results[k]["out_n"] for k in range(NCORES)], axis=0)[
        None, :, :
    ]
    return out, out_norm


def kernel(out1, out2, W, bias):
    from concourse.bass_utils import run_bass_kernel_spmd

    nc = _get_program()
    in_maps = make_in_maps(
        np.asarray(out1), np.asarray(out2), np.asarray(W), np.asarray(bias)
    )
    res = run_bass_kernel_spmd(nc, in_maps, list(range(NCORES)))
    return gather_results(res.results)
